# revision 1
# baseline (speedup 1.0000x reference)
"""Causal self-attention (B=1, S=4096, D=768, H=12) on 8 TRN2 NeuronCores.

Sharding: 4 head-groups (3 heads each) x 2 query-parity halves; no
collectives. Core c = 2*g + p handles heads [3g, 3g+3) and query rows
{r : r % 2 == p} (strided assignment balances causal work perfectly).

Per core:
  - K^T, V for its 3 heads over the full sequence (projected from x^T),
    Q^T for its strided query half (host supplies x^T[:, p::2]).
  - Flash-style causal attention with scores computed transposed
    ([k, q] layout) so the PV matmul needs no transposes; the softmax
    denominator comes from a ones-column appended to V; the causal
    "diagonal band" (1024 keys per 512-query tile, due to striding) is
    handled with a multiplicative {0,1} mask supplied by the host.
  - Partial output projection against its 192 rows of Wout.
Host sums the 4 head-group partials per parity, interleaves parities,
and adds bout.

All matmuls run in bf16 (f32 PSUM accumulation); softmax exp in f32.
"""
import os

import numpy as np
import ml_dtypes

import concourse.bass as bass
import concourse.mybir as mybir
import concourse.tile as tile
from concourse import bacc
from concourse.bass_utils import run_bass_kernel_spmd

BF16 = mybir.dt.bfloat16
F32 = mybir.dt.float32
NPBF16 = ml_dtypes.bfloat16

S = 4096          # sequence length
D = 768           # model dim
HD = 64           # head dim
HL = 3            # heads per core
DL = HL * HD      # 192 local qkv cols per core
SQ = S // 2       # 2048 local queries per core
NQT = 4           # q-tiles per core
QTW = 512         # q-tile width (local queries)
NKB = S // 128    # 32 key blocks of 128
NDC = D // 128    # 6 contraction chunks of 128 over D
GRP = 2           # score chunks per exp group (2 PSUM banks)
VW = HD + 1       # V' column stride per head (64 V cols + ones col)
SCALE = HD ** -0.5


def build_nc():
    nc = bacc.Bacc(None, target_bir_lowering=False)
    xT = nc.declare_dram_parameter("xT", [D, S], BF16, isOutput=False)
    xqT = nc.declare_dram_parameter("xqT", [D, SQ], BF16, isOutput=False)
    wk = nc.declare_dram_parameter("wk", [D, DL], BF16, isOutput=False)
    wq = nc.declare_dram_parameter("wq", [D, DL], BF16, isOutput=False)
    wv = nc.declare_dram_parameter("wv", [D, DL], BF16, isOutput=False)
    bk = nc.declare_dram_parameter("bk", [DL, 1], F32, isOutput=False)
    bq = nc.declare_dram_parameter("bq", [DL, 1], F32, isOutput=False)
    bv = nc.declare_dram_parameter("bv", [DL], F32, isOutput=False)
    wout = nc.declare_dram_parameter("wout", [DL, D], BF16, isOutput=False)
    maskT = nc.declare_dram_parameter("maskT", [1024, QTW], BF16, isOutput=False)
    out = nc.declare_dram_parameter("out", [SQ, D], F32, isOutput=True)

    from contextlib import ExitStack

    with tile.TileContext(nc) as tc, ExitStack() as ctx:
        # ---- all pools persistent & role-separate: no mid-kernel pool
        #      churn (false zone deps) and no cross-role slot contention ----
        persist = ctx.enter_context(tc.tile_pool(name="persist", bufs=1))
        xtp = ctx.enter_context(tc.tile_pool(name="xtp", bufs=1))
        wp = ctx.enter_context(tc.tile_pool(name="wp", bufs=1))
        pp = ctx.enter_context(tc.tile_pool(name="pp", bufs=1, space="PSUM"))
        pvp = ctx.enter_context(tc.tile_pool(name="pvp", bufs=1, space="PSUM"))
        psp = ctx.enter_context(tc.tile_pool(name="psp", bufs=2, space="PSUM"))
        pop = ctx.enter_context(tc.tile_pool(name="pop", bufs=1, space="PSUM"))
        ep = ctx.enter_context(tc.tile_pool(name="ep", bufs=3))
        emp = ctx.enter_context(tc.tile_pool(name="emp", bufs=4))
        rp = ctx.enter_context(tc.tile_pool(name="rp", bufs=2))
        osb = ctx.enter_context(tc.tile_pool(name="osb", bufs=3))

        kT01 = persist.tile([128, S], BF16)         # K^T heads 0,1
        kT2 = persist.tile([64, S], BF16)           # K^T head 2
        qT01 = persist.tile([128, SQ], BF16)        # Q^T heads 0,1
        qT2 = persist.tile([64, SQ], BF16)
        aT01 = persist.tile([128, SQ], BF16)        # attn^T heads 0,1
        aT2 = persist.tile([64, SQ], BF16)
        vbig = persist.tile([128, NKB * HL * VW], BF16)  # V' blocks [k,195]
        mbig = persist.tile([128, 8, QTW], BF16)    # band masks
        bvb = persist.tile([128, DL], F32)          # bv broadcast over rows
        ones1 = persist.tile([1, 64], BF16)
        bk0 = persist.tile([128, 1], F32)
        bk1 = persist.tile([64, 1], F32)
        bq0 = persist.tile([128, 1], F32)
        bq1 = persist.tile([64, 1], F32)
        wo0 = persist.tile([128, D], BF16)
        wo1 = persist.tile([64, D], BF16)

        nc.vector.memset(vbig, 1.0)
        nc.vector.memset(ones1, 1.0)

        # ---- input DMAs: x^T first (the serially-issued DMA queue would
        #      otherwise delay xt0 behind 18 weight DMAs ~12us) ----
        xt = []
        for i in range(NDC):
            t = xtp.tile([128, S], BF16, name=f"xt{i}")
            nc.sync.dma_start(out=t, in_=xT[i * 128:(i + 1) * 128, :])
            xt.append(t)
        wk_sb, wq_sb, wv_sb = [], [], []
        for nm, dram, lst in (("wk", wk, wk_sb), ("wv", wv, wv_sb),
                              ("wq", wq, wq_sb)):
            for i in range(NDC):
                t = wp.tile([128, DL], BF16, name=f"{nm}{i}")
                nc.sync.dma_start(out=t, in_=dram[i * 128:(i + 1) * 128, :])
                lst.append(t)
        xq = []
        for i in range(NDC):
            t = xtp.tile([128, SQ], BF16, name=f"xq{i}")
            nc.sync.dma_start(out=t, in_=xqT[i * 128:(i + 1) * 128, :])
            xq.append(t)
        nc.sync.dma_start(out=bk0, in_=bk[0:128, :])
        nc.sync.dma_start(out=bk1, in_=bk[128:DL, :])
        nc.sync.dma_start(out=bvb, in_=bv[:].partition_broadcast(128))
        nc.sync.dma_start(out=bq0, in_=bq[0:128, :])
        nc.sync.dma_start(out=bq1, in_=bq[128:DL, :])
        nc.sync.dma_start(out=mbig,
                          in_=maskT.rearrange("(b p) q -> p b q", p=128))
        nc.sync.dma_start(out=wo0, in_=wout[0:128, :])
        nc.sync.dma_start(out=wo1, in_=wout[128:DL, :])

        def kq_proj(dst01, dst2, w_sb, rhs_all, b0, b1, n):
            # out[m, cols n*512..] = sum_d W[d, m] * x^T[d, n*512..]
            nsl = slice(n * 512, (n + 1) * 512)
            for m in range(2):
                mw = 128 if m == 0 else 64
                msl = slice(0, 128) if m == 0 else slice(128, DL)
                ps = pp.tile([128, 512], F32, name="pk", tag="pk")
                for kc in range(NDC):
                    nc.tensor.matmul(
                        ps[:mw, :], lhsT=w_sb[kc][:, msl],
                        rhs=rhs_all[kc][:, nsl],
                        start=(kc == 0), stop=(kc == NDC - 1),
                    )
                dst = dst01 if m == 0 else dst2
                bias = (b0 if m == 0 else b1)
                nc.vector.tensor_scalar_add(
                    out=dst[0:mw, nsl], in0=ps[:mw, :], scalar1=bias[:mw, :])

        def v_proj(kb):
            pv = pvp.tile([128, DL], F32, name="pv", tag="pv")
            ksl = slice(kb * 128, (kb + 1) * 128)
            for kc in range(NDC):
                nc.tensor.matmul(
                    pv, lhsT=xt[kc][:, ksl], rhs=wv_sb[kc],
                    start=(kc == 0), stop=(kc == NDC - 1),
                )
            for h in range(HL):
                voff = kb * HL * VW + h * VW
                nc.vector.tensor_add(
                    out=vbig[:, voff:voff + HD],
                    in0=pv[:, h * HD:(h + 1) * HD],
                    in1=bvb[:, h * HD:(h + 1) * HD],
                )

        heads = (
            (kT01[0:64], qT01[0:64], aT01[0:64]),
            (kT01[64:128], qT01[64:128], aT01[64:128]),
            (kT2[0:64], qT2[0:64], aT2[0:64]),
        )

        def attention(t):
            qsl = slice(t * QTW, (t + 1) * QTW)
            nkb = 8 * (t + 1)
            for h in range(HL):
                kT_h, qT_h, aT_h = heads[h]
                po = pop.tile([VW, 512], F32, name="po", tag="po")
                for kb0 in range(0, nkb, GRP):
                    g = min(GRP, nkb - kb0)
                    ps = psp.tile([128, GRP * 512], F32, name="ps", tag="ps")
                    for gi in range(g):
                        kb = kb0 + gi
                        nc.tensor.matmul(
                            ps[:, gi * 512:(gi + 1) * 512],
                            lhsT=kT_h[:, kb * 128:(kb + 1) * 128],
                            rhs=qT_h[:, qsl],
                            start=True, stop=True,
                        )
                    eT = ep.tile([128, GRP * 512], BF16, name="eT", tag="eT")
                    nc.scalar.activation(
                        out=eT[:, :g * 512], in_=ps[:, :g * 512],
                        func=mybir.ActivationFunctionType.Exp, scale=SCALE)
                    for gi in range(g):
                        kb = kb0 + gi
                        src = eT[:, gi * 512:(gi + 1) * 512]
                        if kb >= 8 * t:          # diagonal band: mask
                            b = kb - 8 * t
                            em = emp.tile([128, 512], BF16, name="em", tag="em")
                            nc.vector.tensor_mul(
                                out=em, in0=src, in1=mbig[:, b, :])
                            src = em
                        voff = kb * HL * VW + h * VW
                        nc.tensor.matmul(
                            po[0:VW, :], lhsT=vbig[:, voff:voff + VW],
                            rhs=src,
                            start=(kb == 0), stop=(kb == nkb - 1),
                            skip_group_check=True,
                        )
                # divide by the softmax sum (row HD of po)
                sums = rp.tile([1, 512], BF16, name="sums", tag="sums")
                nc.vector.tensor_copy(out=sums, in_=po[HD:VW, :])
                pb = psp.tile([128, 512], F32, name="pb", tag="aux1", bufs=1)
                nc.tensor.matmul(pb[0:64, :], lhsT=ones1, rhs=sums,
                                 start=True, stop=True)
                recb = rp.tile([64, 512], F32, name="recb", tag="recb")
                nc.vector.reciprocal_approx_fast(out=recb, in_=pb[0:64, :])
                nc.vector.tensor_mul(
                    out=aT_h[:, qsl], in0=po[0:HD, :], in1=recb)

            # out-projection for this t' (aT for all heads now ready)
            for qt in range(4 * t, 4 * (t + 1)):
                osl = slice(qt * 128, (qt + 1) * 128)
                pot = psp.tile([128, 512], F32, name="pot", tag="aux1", bufs=1)
                ot = osb.tile([128, D], F32, name="ot", tag="ot")
                for ncol in range(2):
                    cw = 512 if ncol == 0 else 256
                    csl = slice(ncol * 512, ncol * 512 + cw)
                    nc.tensor.matmul(
                        pot[:, :cw], lhsT=aT01[:, osl], rhs=wo0[:, csl],
                        start=True, stop=False, skip_group_check=True)
                    nc.tensor.matmul(
                        pot[:, :cw], lhsT=aT2[:, osl], rhs=wo1[:, csl],
                        start=False, stop=True, skip_group_check=True)
                    nc.vector.tensor_copy(out=ot[:, csl], in_=pot[:, :cw])
                nc.gpsimd.dma_start(out=out[osl, :], in_=ot)

        # ---- interleaved schedule: K-proj, then V/Q slices feed each t ----
        for n in range(8):
            kq_proj(kT01, kT2, wk_sb, xt, bk0, bk1, n)
        for t in range(NQT):
            for kb in range(8 * t, 8 * (t + 1)):
                v_proj(kb)
            kq_proj(qT01, qT2, wq_sb, xq, bq0, bq1, t)
            attention(t)

    nc.finalize()
    return nc


_NC_CACHE = {}


def _get_nc():
    if "nc" not in _NC_CACHE:
        _NC_CACHE["nc"] = build_nc()
    return _NC_CACHE["nc"]


def kernel(x, Wqkv, bqkv, Wout, bout):
    x = np.asarray(x, dtype=np.float32)
    Wqkv = np.asarray(Wqkv, dtype=np.float32)
    bqkv = np.asarray(bqkv, dtype=np.float32)
    Wout = np.asarray(Wout, dtype=np.float32)
    bout = np.asarray(bout, dtype=np.float32)
    B, S_, D_ = x.shape
    assert (B, S_, D_) == (1, S, D)
    nc = _get_nc()

    xT_np = np.ascontiguousarray(x[0].T).astype(NPBF16)          # [768, 4096]
    in_maps = []
    for c in range(8):
        g, p = c // 2, c % 2
        csl = slice(DL * g, DL * (g + 1))
        kk = np.arange(1024, dtype=np.int64)[:, None]
        jj = np.arange(QTW, dtype=np.int64)[None, :]
        mask = (kk <= 2 * jj + p).astype(NPBF16)
        in_maps.append({
            "xT": xT_np,
            "xqT": np.ascontiguousarray(xT_np[:, p::2]),
            "wk": np.ascontiguousarray(Wqkv[:, D + DL * g:D + DL * (g + 1)]).astype(NPBF16),
            "wq": np.ascontiguousarray(Wqkv[:, csl]).astype(NPBF16),
            "wv": np.ascontiguousarray(Wqkv[:, 2 * D + DL * g:2 * D + DL * (g + 1)]).astype(NPBF16),
            "bk": np.ascontiguousarray(bqkv[D + DL * g:D + DL * (g + 1)]).astype(np.float32).reshape(DL, 1),
            "bq": np.ascontiguousarray(bqkv[csl]).astype(np.float32).reshape(DL, 1),
            "bv": np.ascontiguousarray(bqkv[2 * D + DL * g:2 * D + DL * (g + 1)]).astype(np.float32),
            "wout": np.ascontiguousarray(Wout[csl, :]).astype(NPBF16),
            "maskT": mask,
        })

    trace = bool(int(os.environ.get("ATTN_TRACE", "0")))
    tmpdir = os.environ.get("ATTN_TMPDIR") or None
    res = run_bass_kernel_spmd(nc, in_maps, core_ids=list(range(8)), trace=trace,
                               tmpdir=tmpdir)
    if trace:
        _NC_CACHE["last_result"] = res

    out_full = np.zeros((S, D), np.float32)
    for p in range(2):
        acc = np.zeros((SQ, D), np.float32)
        for g in range(4):
            acc += res.results[2 * g + p]["out"]
        out_full[p::2] = acc
    out_full += bout.astype(np.float32)[None, :]
    return out_full[None].astype(np.float32)



# revision 3
# speedup vs baseline: 1.1734x; 1.1734x over previous
"""Causal self-attention (B=1, S=4096, D=768, H=12) on 8 TRN2 NeuronCores.

Sharding: 4 head-groups (3 heads each) x 2 query-parity halves; no
collectives. Core c = 2*g + p handles heads [3g, 3g+3) and query rows
{r : r % 2 == p} (strided assignment balances causal work perfectly).

V2 vs baseline:
  - DMA head killed: inputs arrive as few large need-ordered DMAs
    (weights first, x^T in 512-column slices) so the PE starts at ~4us
    instead of ~40us.
  - Causal band truncation: the 8 diagonal blocks per q-tile only
    compute scores/exp/PV on their causally-valid query range; band
    blocks are packed in pairs (1,7),(2,6),(3,5) so each EXP call still
    covers a contiguous 512-col region (ACT has ~300ns/call overhead).
  - The {0,1} mask shrinks to a single [128,64] triangle applied
    in-place to the 64 partial columns of each band block.
  - Independent PE work (next tile's V/Q projections, previous tile's
    out-projection) is interleaved into the EXP-paced attention stream
    so the tensor engine never idles behind the scalar engine.

All matmuls run in bf16 (f32 PSUM accumulation); softmax exp in f32.
"""
import os

import numpy as np
import ml_dtypes

import concourse.bass as bass
import concourse.mybir as mybir
import concourse.tile as tile
from concourse import bacc
from concourse.bass_utils import run_bass_kernel_spmd

BF16 = mybir.dt.bfloat16
F32 = mybir.dt.float32
NPBF16 = ml_dtypes.bfloat16

S = 4096          # sequence length
D = 768           # model dim
HD = 64           # head dim
HL = 3            # heads per core
DL = HL * HD      # 192 local qkv cols per core
SQ = S // 2       # 2048 local queries per core
NQT = 4           # q-tiles per core
QTW = 512         # q-tile width (local queries)
NKB = S // 128    # 32 key blocks of 128
NDC = D // 128    # 6 contraction chunks of 128 over D
VW = HD + 1       # V' column stride per head (64 V cols + ones col)
SCALE = HD ** -0.5

# band packs: diagonal blocks b paired so each pack's widths sum to <=512
BAND_PACKS = ((0,), (1, 7), (2, 6), (3, 5), (4,))


def build_nc():
    nc = bacc.Bacc(None, target_bir_lowering=False)
    xT = nc.declare_dram_parameter("xT", [D, S], BF16, isOutput=False)
    xqT = nc.declare_dram_parameter("xqT", [D, SQ], BF16, isOutput=False)
    wk = nc.declare_dram_parameter("wk", [D, DL], BF16, isOutput=False)
    wq = nc.declare_dram_parameter("wq", [D, DL], BF16, isOutput=False)
    wv = nc.declare_dram_parameter("wv", [D, DL], BF16, isOutput=False)
    bkq = nc.declare_dram_parameter("bkq", [DL, 2], F32, isOutput=False)
    bv = nc.declare_dram_parameter("bv", [DL], F32, isOutput=False)
    wout = nc.declare_dram_parameter("wout", [DL, D], BF16, isOutput=False)
    mask64 = nc.declare_dram_parameter("mask64", [128, 64], BF16, isOutput=False)
    out = nc.declare_dram_parameter("out", [SQ, D], F32, isOutput=True)

    from contextlib import ExitStack

    with tile.TileContext(nc) as tc, ExitStack() as ctx:
        persist = ctx.enter_context(tc.tile_pool(name="persist", bufs=1))
        xtp = ctx.enter_context(tc.tile_pool(name="xtp", bufs=1))
        wp = ctx.enter_context(tc.tile_pool(name="wp", bufs=1))
        pp = ctx.enter_context(tc.tile_pool(name="pp", bufs=1, space="PSUM"))
        pvp = ctx.enter_context(tc.tile_pool(name="pvp", bufs=1, space="PSUM"))
        psp = ctx.enter_context(tc.tile_pool(name="psp", bufs=2, space="PSUM"))
        pop = ctx.enter_context(tc.tile_pool(name="pop", bufs=1, space="PSUM"))
        ep = ctx.enter_context(tc.tile_pool(name="ep", bufs=3))
        rp = ctx.enter_context(tc.tile_pool(name="rp", bufs=2))
        osb = ctx.enter_context(tc.tile_pool(name="osb", bufs=3))

        kT01 = persist.tile([128, S], BF16)         # K^T heads 0,1
        kT2 = persist.tile([64, S], BF16)           # K^T head 2
        qT01 = persist.tile([128, SQ], BF16)        # Q^T heads 0,1
        qT2 = persist.tile([64, SQ], BF16)
        aT01 = persist.tile([128, SQ], BF16)        # attn^T heads 0,1
        aT2 = persist.tile([64, SQ], BF16)
        vbig = persist.tile([128, NKB * HL * VW], BF16)  # V' blocks [k,195]
        bvb = persist.tile([128, DL], F32)          # bv broadcast over rows
        msk = persist.tile([128, 64], BF16)         # causal triangle r<=2c+p
        ones1 = persist.tile([1, 64], BF16)
        bkq0 = persist.tile([128, 2], F32)
        bkq1 = persist.tile([64, 2], F32)
        wo0 = persist.tile([128, D], BF16)
        wo1 = persist.tile([64, D], BF16)

        nc.vector.memset(vbig, 1.0)
        nc.vector.memset(ones1, 1.0)

        # x^T / xq^T land as 512-column slices holding all 6 contraction
        # chunks: tile cols = kc*512 + j. Weights land as [128, 6*DL].
        xt = [xtp.tile([128, NDC * 512], BF16, name=f"xt{n}") for n in range(8)]
        xq = [xtp.tile([128, NDC * 512], BF16, name=f"xq{t}") for t in range(NQT)]
        wk_t = wp.tile([128, NDC * DL], BF16, name="wk")
        wq_t = wp.tile([128, NDC * DL], BF16, name="wq")
        wv_t = wp.tile([128, NDC * DL], BF16, name="wv")

        xT_r = xT.rearrange("(c p) n -> p c n", p=128)      # [128, 6, 4096]
        xq_r = xqT.rearrange("(c p) n -> p c n", p=128)     # [128, 6, 2048]

        def dma_x(dst, src_r, j0):
            nc.sync.dma_start(
                out=dst.rearrange("p (c n) -> p c n", n=512),
                in_=src_r[:, :, j0:j0 + 512])

        # need-ordered input DMAs (sync queue ~0.7us issue each):
        nc.sync.dma_start(out=wk_t.rearrange("p (c m) -> p c m", m=DL),
                          in_=wk.rearrange("(c p) m -> p c m", p=128))
        dma_x(xt[0], xT_r, 0)
        dma_x(xt[1], xT_r, 512)
        dma_x(xt[2], xT_r, 1024)
        dma_x(xt[3], xT_r, 1536)
        nc.sync.dma_start(out=wq_t.rearrange("p (c m) -> p c m", m=DL),
                          in_=wq.rearrange("(c p) m -> p c m", p=128))
        nc.sync.dma_start(out=wv_t.rearrange("p (c m) -> p c m", m=DL),
                          in_=wv.rearrange("(c p) m -> p c m", p=128))
        dma_x(xq[0], xq_r, 0)
        dma_x(xt[4], xT_r, 2048)
        dma_x(xt[5], xT_r, 2560)
        dma_x(xt[6], xT_r, 3072)
        dma_x(xt[7], xT_r, 3584)
        dma_x(xq[1], xq_r, 512)
        dma_x(xq[2], xq_r, 1024)
        dma_x(xq[3], xq_r, 1536)
        # small tensors on the gpsimd queue (parallel issue path)
        nc.gpsimd.dma_start(out=bkq0, in_=bkq[0:128, :])
        nc.gpsimd.dma_start(out=bkq1, in_=bkq[128:DL, :])
        nc.gpsimd.dma_start(out=bvb, in_=bv[:].partition_broadcast(128))
        nc.gpsimd.dma_start(out=msk, in_=mask64[:, :])
        nc.gpsimd.dma_start(out=wo0, in_=wout[0:128, :])
        nc.gpsimd.dma_start(out=wo1, in_=wout[128:DL, :])

        def kq_proj(dst01, dst2, w_t, rhs, bc, n, m):
            # dst[m-rows, cols n*512..] = W^T x^T + b  for one m-pass
            nsl = slice(n * 512, (n + 1) * 512)
            mw = 128 if m == 0 else 64
            msl = slice(0, 128) if m == 0 else slice(128, DL)
            ps = pp.tile([128, 512], F32, name="pk", tag="pk")
            for kc in range(NDC):
                nc.tensor.matmul(
                    ps[:mw, :],
                    lhsT=w_t[:, kc * DL:(kc + 1) * DL][:, msl],
                    rhs=rhs[:, kc * 512:(kc + 1) * 512],
                    start=(kc == 0), stop=(kc == NDC - 1),
                )
            dst = dst01 if m == 0 else dst2
            bias = (bkq0 if m == 0 else bkq1)[:, bc:bc + 1]
            nc.vector.tensor_scalar_add(
                out=dst[0:mw, nsl], in0=ps[:mw, :], scalar1=bias[:mw, :])

        def v_proj(kb):
            pv = pvp.tile([128, DL], F32, name="pv", tag="pv")
            n, j = kb // 4, (kb % 4) * 128
            for kc in range(NDC):
                nc.tensor.matmul(
                    pv, lhsT=xt[n][:, kc * 512 + j:kc * 512 + j + 128],
                    rhs=wv_t[:, kc * DL:(kc + 1) * DL],
                    start=(kc == 0), stop=(kc == NDC - 1),
                )
            # one strided add writes all 3 heads' V cols (ones col skipped)
            voff = kb * HL * VW
            dstv = vbig[:, voff:voff + HL * VW]
            dstv = dstv.rearrange("p (h vw) -> p h vw", vw=VW)[:, :, 0:HD]
            nc.vector.tensor_add(
                out=dstv,
                in0=pv.rearrange("p (h d) -> p h d", d=HD),
                in1=bvb.rearrange("p (h d) -> p h d", d=HD),
            )

        heads = (
            (kT01[0:64], qT01[0:64], aT01[0:64]),
            (kT01[64:128], qT01[64:128], aT01[64:128]),
            (kT2[0:64], qT2[0:64], aT2[0:64]),
        )

        def out_proj(qt):
            osl = slice(qt * 128, (qt + 1) * 128)
            pot = psp.tile([128, 512], F32, name="pot", tag="aux1", bufs=1)
            ot = osb.tile([128, D], F32, name="ot", tag="ot")
            for ncol in range(2):
                cw = 512 if ncol == 0 else 256
                csl = slice(ncol * 512, ncol * 512 + cw)
                nc.tensor.matmul(
                    pot[:, :cw], lhsT=aT01[:, osl], rhs=wo0[:, csl],
                    start=True, stop=False, skip_group_check=True)
                nc.tensor.matmul(
                    pot[:, :cw], lhsT=aT2[:, osl], rhs=wo1[:, csl],
                    start=False, stop=True, skip_group_check=True)
                nc.vector.tensor_copy(out=ot[:, csl], in_=pot[:, :cw])
            nc.gpsimd.dma_start(out=out[osl, :], in_=ot)

        def attention(t, fillers):
            def pump(k=1):
                for _ in range(k):
                    if fillers:
                        fillers.pop(0)()

            qoff = t * QTW
            # pack list: (kb, psum_off, width, q_start, is_band)
            packs = []
            for kb0 in range(0, 8 * t, 2):
                packs.append([(kb0, 0, 512, 0, False),
                              (kb0 + 1, 512, 512, 0, False)])
            for pr in BAND_PACKS:
                lst, off = [], 0
                for b in pr:
                    w = 512 - 64 * b
                    lst.append((8 * t + b, off, w, 64 * b, True))
                    off += w
                packs.append(lst)
            last_kb = 8 * t + BAND_PACKS[-1][-1]

            for h in range(HL):
                kT_h, qT_h, aT_h = heads[h]
                po = pop.tile([VW, 512], F32, name="po", tag="po")
                for pack in packs:
                    tw = sum(p[2] for p in pack)
                    ps = psp.tile([128, 1024], F32, name="ps", tag="ps")
                    for (kb, off, w, qs, _band) in pack:
                        nc.tensor.matmul(
                            ps[:, off:off + w],
                            lhsT=kT_h[:, kb * 128:(kb + 1) * 128],
                            rhs=qT_h[:, qoff + qs:qoff + QTW],
                            start=True, stop=True,
                        )
                    eT = ep.tile([128, 1024], BF16, name="eT", tag="eT")
                    nc.scalar.activation(
                        out=eT[:, :tw], in_=ps[:, :tw],
                        func=mybir.ActivationFunctionType.Exp, scale=SCALE)
                    for (kb, off, w, qs, band) in pack:
                        if band:  # zero the 64 partial cols of the triangle
                            nc.vector.tensor_mul(
                                out=eT[:, off:off + 64],
                                in0=eT[:, off:off + 64], in1=msk)
                        voff = kb * HL * VW + h * VW
                        nc.tensor.matmul(
                            po[0:VW, qs:QTW], lhsT=vbig[:, voff:voff + VW],
                            rhs=eT[:, off:off + w],
                            start=(kb == 0), stop=(kb == last_kb),
                            skip_group_check=True,
                        )
                    pump(1)
                # divide by the softmax sum (row HD of po)
                sums = rp.tile([1, 512], BF16, name="sums", tag="sums")
                nc.vector.tensor_copy(out=sums, in_=po[HD:VW, :])
                pb = psp.tile([128, 512], F32, name="pb", tag="aux1", bufs=1)
                nc.tensor.matmul(pb[0:64, :], lhsT=ones1, rhs=sums,
                                 start=True, stop=True)
                recb = rp.tile([64, 512], F32, name="recb", tag="recb")
                nc.vector.reciprocal_approx_fast(out=recb, in_=pb[0:64, :])
                nc.vector.tensor_mul(
                    out=aT_h[:, qoff:qoff + QTW], in0=po[0:HD, :], in1=recb)
                pump(1)
            pump(len(fillers))

        # ---- schedule ----
        for n in range(8):
            for m in range(2):
                kq_proj(kT01, kT2, wk_t, xt[n], 0, n, m)
        for kb in range(8):
            v_proj(kb)
        for m in range(2):
            kq_proj(qT01, qT2, wq_t, xq[0], 1, 0, m)

        for t in range(NQT):
            fillers = []
            if t < NQT - 1:
                for kb in range(8 * (t + 1), 8 * (t + 2)):
                    fillers.append(lambda kb=kb: v_proj(kb))
                for m in range(2):
                    fillers.append(
                        lambda t=t, m=m: kq_proj(qT01, qT2, wq_t, xq[t + 1],
                                                 1, t + 1, m))
            if t > 0:
                for qt in range(4 * (t - 1), 4 * t):
                    fillers.append(lambda qt=qt: out_proj(qt))
            attention(t, fillers)
        for qt in range(4 * (NQT - 1), 4 * NQT):
            out_proj(qt)

    nc.finalize()
    return nc


_NC_CACHE = {}


def _get_nc():
    if "nc" not in _NC_CACHE:
        _NC_CACHE["nc"] = build_nc()
    return _NC_CACHE["nc"]


def kernel(x, Wqkv, bqkv, Wout, bout):
    x = np.asarray(x, dtype=np.float32)
    Wqkv = np.asarray(Wqkv, dtype=np.float32)
    bqkv = np.asarray(bqkv, dtype=np.float32)
    Wout = np.asarray(Wout, dtype=np.float32)
    bout = np.asarray(bout, dtype=np.float32)
    B, S_, D_ = x.shape
    assert (B, S_, D_) == (1, S, D)
    nc = _get_nc()

    xT_np = np.ascontiguousarray(x[0].T).astype(NPBF16)          # [768, 4096]
    in_maps = []
    for c in range(8):
        g, p = c // 2, c % 2
        csl = slice(DL * g, DL * (g + 1))
        rr = np.arange(128, dtype=np.int64)[:, None]
        cc = np.arange(64, dtype=np.int64)[None, :]
        mask = (rr <= 2 * cc + p).astype(NPBF16)
        bk_h = bqkv[D + DL * g:D + DL * (g + 1)].astype(np.float32)
        bq_h = bqkv[csl].astype(np.float32)
        in_maps.append({
            "xT": xT_np,
            "xqT": np.ascontiguousarray(xT_np[:, p::2]),
            "wk": np.ascontiguousarray(Wqkv[:, D + DL * g:D + DL * (g + 1)]).astype(NPBF16),
            "wq": np.ascontiguousarray(Wqkv[:, csl]).astype(NPBF16),
            "wv": np.ascontiguousarray(Wqkv[:, 2 * D + DL * g:2 * D + DL * (g + 1)]).astype(NPBF16),
            "bkq": np.ascontiguousarray(np.stack([bk_h, bq_h], axis=1)),
            "bv": np.ascontiguousarray(bqkv[2 * D + DL * g:2 * D + DL * (g + 1)]).astype(np.float32),
            "wout": np.ascontiguousarray(Wout[csl, :]).astype(NPBF16),
            "mask64": mask,
        })

    trace = bool(int(os.environ.get("ATTN_TRACE", "0")))
    tmpdir = os.environ.get("ATTN_TMPDIR") or None
    res = run_bass_kernel_spmd(nc, in_maps, core_ids=list(range(8)), trace=trace,
                               tmpdir=tmpdir)
    if trace:
        _NC_CACHE["last_result"] = res

    out_full = np.zeros((S, D), np.float32)
    for p in range(2):
        acc = np.zeros((SQ, D), np.float32)
        for g in range(4):
            acc += res.results[2 * g + p]["out"]
        out_full[p::2] = acc
    out_full += bout.astype(np.float32)[None, :]
    return out_full[None].astype(np.float32)
